# revision 60
# baseline (speedup 1.0000x reference)
# AttnPool1dWindow Trainium2 kernel (v3: fp8 DoubleRow phase-1, on-chip
# token-layout softmax, scaled-stationary band matmuls for the windowed sum).
# B=8, C=512, L=4096, kernel_size=16, stride=8, P=511. One batch per core.
import numpy as np

B, C, L = 8, 512, 4096
WIN, ST = 16, 8
P = 1 + (L - WIN) // ST          # 511
NCH = 8                           # token chunks of 512
NPT = 4                           # p tiles of 128
NTT = 32                          # token tiles of 128

# cf32 blob column offsets
CF_BCOL = 0          # [128, 4]   bias per d-tile
CF_NOTMT = 4         # [128, 32]  token-layout not-masked
CF_E8X0 = 36         # [16, 128]  expand den -> token (q == r//8)
CF_E8X1 = 164        # [16, 128]  expand den -> token (q == r//8 - 1)
CF_E8P = 292         # [16, 128]  boundary expand (q==15 & r<8)
CF_S8W = 420         # [128, 16]  window sum (r//8 in {q, q-1})
CF_S8TP = 436        # [128, 16]  boundary window sum (q==15 & r<8)
CF_EYE = 452         # [1, 1]     1.0 (transpose identity)
CF_W = 453

# cbf16 blob column offsets
CB_VCOL = 0          # [128, 4]   v per d-tile
CB_E0BX = 4          # [128, 17]  [0 | j == r//8] (17-wide band + strip)
CB_E1BX = 21         # [128, 17]  [r<8 | j == r//8 - 1]
CB_W = 38

# misc psum bank regions (f32 cols; 512B zero-zones)
MI_ST = 0            # sT        [128, 32]
MI_DEN = 128         # den       [16, 8]   window sums for k
MI_RD0 = 256         # rdent0    [128, 8]
MI_RD1 = 384         # rden1t    [128, 9]

_CACHE = {}


def _custom_ap(ap, dims, extra_offset=0):
    """Copy of `ap` with explicit [step, count] dims (element units)."""
    import bass_rust
    c = ap.copy()
    c.ap = bass_rust.VecI64Pair(dims)
    if extra_offset:
        c.offset = c.offset + extra_offset
    return c


def _build_bass():
    import concourse.bacc as bacc
    import concourse.mybir as mybir
    import concourse.tile as tile
    dt = mybir.dt
    f32, bf16, f8 = dt.float32, dt.bfloat16, dt.float8e4
    AF = mybir.ActivationFunctionType
    ALU = mybir.AluOpType
    DR = mybir.MatmulPerfMode.DoubleRow

    nc = bacc.Bacc("TRN2", target_bir_lowering=False, debug=False, num_devices=8)

    xf8_d = nc.declare_dram_parameter("xf8", [128, NCH * 4 * 512], f8, isOutput=False)
    xt_d = nc.declare_dram_parameter("xt", [128, NTT * 512], bf16, isOutput=False)
    w8_d = nc.declare_dram_parameter("w8", [128, 4 * 512], f8, isOutput=False)
    cf32_d = nc.declare_dram_parameter("cf32", [128, CF_W], f32, isOutput=False)
    cbf16_d = nc.declare_dram_parameter("cbf16", [128, CB_W], bf16, isOutput=False)
    out_d = nc.declare_dram_parameter("outt", [512, C], f32, isOutput=True)

    with tile.TileContext(nc) as tc:
        with (
            tc.tile_pool(name="big", bufs=1) as big,
            tc.tile_pool(name="hx", bufs=6) as hx,
            tc.tile_pool(name="smk", bufs=4) as smk,
            tc.tile_pool(name="outs", bufs=2) as outs,
            tc.tile_pool(name="ypsum", bufs=3, space="PSUM") as ypsum,
            tc.tile_pool(name="spsum", bufs=1, space="PSUM") as spsum,
            tc.tile_pool(name="mpsum", bufs=1, space="PSUM") as mpsum,
            tc.tile_pool(name="opsum", bufs=2, space="PSUM") as opsum,
        ):
            xf8 = big.tile([128, 4, L], f8, tag="xf8")
            xt = big.tile([128, NTT, C], bf16, tag="xt")
            w8 = big.tile([128, 4, C], f8, tag="w8")
            cf32 = big.tile([128, CF_W], f32, tag="cf32")
            cbf16 = big.tile([128, CB_W], bf16, tag="cbf16")
            s8 = big.tile([128, 1536], f32, tag="s8")
            ewm = big.tile([128, 33], f32, tag="ewm")
            rdenp = big.tile([16, 35], f32, tag="rdenp")
            u0t = big.tile([128, 8], f32, tag="u0t")
            u1t = big.tile([128, 9], f32, tag="u1t")
            ebandsAB = [
                [
                    big.tile([128, 128], bf16, tag=f"eb{s}{d}",
                             name=f"eband{s}{d}")
                    for d in range(9)
                ]
                for s in range(2)
            ]

            misc = mpsum.tile([128, 512], f32, tag="MISC")
            sp_cur = {}

            # ---- loads (sync engine / HWDGE) ----
            nc.sync.dma_start(out=w8[:, :, :], in_=w8_d[:, :])

            def load_xf8(li):
                nc.sync.dma_start(
                    out=xf8[:, :, 512 * li:512 * li + 512],
                    in_=_custom_ap(xf8_d[:], [[NCH * 4 * 512, 128], [512, 4], [1, 512]],
                                   2048 * li),
                )

            def load_xt(g):  # 8 token tiles per group
                nc.sync.dma_start(
                    out=xt[:, 8 * g:8 * g + 8, :],
                    in_=xt_d[:, 4096 * g:4096 * g + 4096],
                )

            load_xf8(0)
            nc.sync.dma_start(out=cf32[:, :], in_=cf32_d[:, :])
            load_xf8(1)
            nc.sync.dma_start(out=cbf16[:, :], in_=cbf16_d[:, :])
            for li in range(2, NCH):
                load_xf8(li)
            for g in range(4):
                load_xt(g)

            # ---- one-time zeroing ----
            for s in range(2):
                for d in range(9):
                    nc.vector.memset(ebandsAB[s][d][:, :], 0.0)
            nc.vector.memset(rdenp[:, :], 0.0)
            nc.vector.memset(ewm[:, :], 0.0)

            warm = big.tile([1, 4], f32, tag="warm")
            nc.vector.memset(warm[0:1, :], 0.0)
            wo = smk.tile([1, 4], f32, tag="WO")
            nc.scalar.activation(wo[0:1, :], warm[0:1, :], AF.Tanh)
            # PE p-state warmup: ~3us of dummy matmuls on zeroed tiles while
            # the first input DMAs are in flight
            ypw = ypsum.tile([128, 128], f32, tag="Y", name="ypwarm")
            for _ in range(16):
                nc.tensor.matmul(ypw[:, :], ebandsAB[0][0][:, :],
                                 ebandsAB[1][0][:, :], start=True, stop=True)

            htiles = {}

            def w_chunk(li, dis=range(4)):
                """Phase-1 projection matmuls + tanh for chunk li."""
                sl = slice(512 * li, 512 * li + 512)
                for di in dis:
                    yp = ypsum.tile([128, 512], f32, tag="Y")
                    nc.tensor.matmul(
                        yp[:, :], w8[:, 0:2, 128 * di:128 * di + 128],
                        xf8[:, 0:2, sl], start=True, stop=False, perf_mode=DR)
                    nc.tensor.matmul(
                        yp[:, :], w8[:, 2:4, 128 * di:128 * di + 128],
                        xf8[:, 2:4, sl], start=False, stop=True, perf_mode=DR)
                    h = hx.tile([128, 512], bf16, tag="H")
                    nc.scalar.activation(h[:, :], yp[:, :], AF.Tanh,
                                         bias=cf32[:, CF_BCOL + di:CF_BCOL + di + 1])
                    htiles[(li, di)] = h

            def v_chunk(li, copy_act=False, inter=None):
                """v.h into a 32-aligned psum row; s -> SBUF; transpose; exp.
                `inter`: callables emitted after each v-matmul to fill the
                tanh-stagger gaps with independent PE work."""
                if li % 3 == 0:
                    sp_cur[0] = spsum.tile([128, 512], f32, tag="SP", name="sp")
                bank = sp_cur[0]
                row = 32 * (li % 3)
                for di in range(4):
                    nc.tensor.matmul(
                        bank[row:row + 1, :],
                        cbf16[:, CB_VCOL + di:CB_VCOL + di + 1],
                        htiles.pop((li, di))[:, :],
                        start=(di == 0), stop=(di == 3))
                    if inter is not None:
                        inter[di]()
                sc = 512 * (li // 3)
                if copy_act:
                    nc.scalar.copy(s8[row:row + 1, sc:sc + 512],
                                   bank[row:row + 1, :])
                else:
                    nc.vector.tensor_copy(s8[row:row + 1, sc:sc + 512],
                                          bank[row:row + 1, :])
                eye1 = cf32[row:row + 1, CF_EYE:CF_EYE + 1]
                for j in range(4):
                    nc.tensor.transpose(
                        misc[:, MI_ST + 4 * li + j:MI_ST + 4 * li + j + 1],
                        s8[row:row + 1, sc + 128 * j:sc + 128 * j + 128], eye1)
                if li % 2 == 1 and li != 7:
                    return          # exp deferred to the next (even) chunk
                lo = 4 * li if li in (0, 7) else 4 * (li - 1)
                hi = 4 * li + 4
                ew4 = smk.tile([128, 8], f32, tag="EW4")
                nc.scalar.activation(ew4[:, 0:hi - lo],
                                     misc[:, MI_ST + lo:MI_ST + hi], AF.Exp)
                nc.vector.tensor_mul(
                    ewm[:, lo:hi], ew4[:, 0:hi - lo],
                    cf32[:, CF_NOTMT + lo:CF_NOTMT + hi])

            def p2_pre(k, m0=0, m1=8):
                """Columns [m0, m1) of k's den/expand/u/band chain. Columns
                0..2 depend only on chunk 2k (boundary terms included), so an
                (0,3) call can run a chunk earlier than the (3,8) remainder."""
                c0 = 8 * k
                ebands = ebandsAB[k % 2]
                nd = 9 if k < 3 else 8
                u1hi = 9 if m1 >= 8 else m1
                nc.tensor.matmul(
                    misc[0:16, MI_DEN + m0:MI_DEN + m1],
                    cf32[:, CF_S8W:CF_S8W + 16], ewm[:, c0 + m0:c0 + m1],
                    start=True, stop=False)
                nc.tensor.matmul(
                    misc[0:16, MI_DEN + m0:MI_DEN + m1],
                    cf32[:, CF_S8TP:CF_S8TP + 16],
                    ewm[:, c0 + m0 + 1:c0 + m1 + 1],
                    start=False, stop=True)
                dcl = smk.tile([16, 8], f32, tag="DCL")
                nc.vector.tensor_scalar_max(dcl[:, 0:m1 - m0],
                                            misc[0:16, MI_DEN + m0:MI_DEN + m1],
                                            1e-6)
                nc.vector.reciprocal(rdenp[:, 1 + c0 + m0:1 + c0 + m1],
                                     dcl[:, 0:m1 - m0])
                # expand den back to token layout (PE)
                nc.tensor.matmul(misc[:, MI_RD0 + m0:MI_RD0 + m1],
                                 cf32[0:16, CF_E8X0:CF_E8X0 + 128],
                                 rdenp[:, 1 + c0 + m0:1 + c0 + m1],
                                 start=True, stop=True)
                nc.tensor.matmul(misc[:, MI_RD1 + m0:MI_RD1 + u1hi],
                                 cf32[0:16, CF_E8X1:CF_E8X1 + 128],
                                 rdenp[:, 1 + c0 + m0:1 + c0 + u1hi],
                                 start=True, stop=False)
                nc.tensor.matmul(misc[:, MI_RD1 + m0:MI_RD1 + u1hi],
                                 cf32[0:16, CF_E8P:CF_E8P + 128],
                                 rdenp[:, c0 + m0:c0 + u1hi],
                                 start=False, stop=True)
                nc.vector.tensor_mul(u0t[:, m0:m1], ewm[:, c0 + m0:c0 + m1],
                                     misc[:, MI_RD0 + m0:MI_RD0 + m1])
                nc.vector.tensor_mul(u1t[:, m0:u1hi], ewm[:, c0 + m0:c0 + u1hi],
                                     misc[:, MI_RD1 + m0:MI_RD1 + u1hi])
                # bands: 17-wide writes cover the band plus the strip column
                # (rows 0..7 of tile d feed the previous window 16d-1)
                for d in range(m0, min(m1, 8)):
                    w0 = 1 if d == 0 else 0    # d=0 has no strip column
                    lo = 16 * d - 1 + w0
                    tmp = smk.tile([128, 17], bf16, tag="TMP")
                    nc.vector.tensor_scalar_mul(
                        tmp[:, w0:17], cbf16[:, CB_E1BX + w0:CB_E1BX + 17],
                        u1t[:, d:d + 1])
                    nc.vector.scalar_tensor_tensor(
                        ebands[d][:, lo:16 * d + 16],
                        cbf16[:, CB_E0BX + w0:CB_E0BX + 17], u0t[:, d:d + 1],
                        tmp[:, w0:17], ALU.mult, ALU.add)
                if m1 >= 8 and nd == 9:
                    nc.vector.tensor_scalar_mul(
                        ebands[8][:, 127:128], cbf16[:, CB_E1BX:CB_E1BX + 1],
                        u1t[:, 8:9])

            p2_ops = {}

            def p2_mm(k, dlo=0, dhi=9):
                c0 = 8 * k
                ebands = ebandsAB[k % 2]
                nd = 9 if k < 3 else 8
                dhi = min(dhi, nd)
                if dlo == 0:
                    p2_ops[k] = opsum.tile([128, 512], f32, tag="OP", name="op")
                op = p2_ops[k]
                for d in range(dlo, dhi):
                    nc.tensor.matmul(op[:, :], ebands[d][:, :], xt[:, c0 + d, :],
                                     start=(d == 0), stop=(d == nd - 1))
                if dhi < nd:
                    return
                ot = outs.tile([128, 512], f32, tag="OT")
                nc.vector.tensor_copy(ot[:, :], op[:, :])
                nc.sync.dma_start(out=out_d[128 * k:128 * k + 128, :], in_=ot[:, :])

            def phase2(k):
                p2_pre(k)
                p2_mm(k)

            w_chunk(0)
            w_chunk(1)
            v_chunk(0)
            w_chunk(2)
            v_chunk(1)
            w_chunk(3)
            v_chunk(2)
            phase2(0)
            w_chunk(4)
            v_chunk(3)
            w_chunk(5)
            v_chunk(4)
            phase2(1)
            w_chunk(6)
            v_chunk(5)
            w_chunk(7, dis=(0, 1))
            v_chunk(6)
            w_chunk(7, dis=(2, 3))
            p2_pre(2)
            v_chunk(7, copy_act=True)
            p2_mm(2, 0, 3)
            p2_pre(3)
            p2_mm(2, 3, 9)
            p2_mm(3)
    nc.compile()
    return nc


def _build_host_constants(mask_b):
    """cf32/cbf16 blobs for one batch (mask-dependent)."""
    import ml_dtypes
    bf16 = ml_dtypes.bfloat16
    r = np.arange(128)
    q16 = np.arange(16)

    cf32 = np.zeros((128, CF_W), np.float32)
    cbf16 = np.zeros((128, CB_W), np.float32)

    # cf32: notmt, E8X0/E8X1/E8P, S8T, eye
    cf32[:, CF_NOTMT:CF_NOTMT + 32] = (~mask_b).astype(np.float32).reshape(32, 128).T
    e8x0 = (np.arange(16)[:, None] == r[None, :] // 8).astype(np.float32)
    e8x1 = (np.arange(16)[:, None] == r[None, :] // 8 - 1).astype(np.float32)
    e8p = ((np.arange(16)[:, None] == 15) & (r[None, :] < 8)).astype(np.float32)
    cf32[0:16, CF_E8X0:CF_E8X0 + 128] = e8x0
    cf32[0:16, CF_E8X1:CF_E8X1 + 128] = e8x1
    cf32[0:16, CF_E8P:CF_E8P + 128] = e8p
    cf32[:, CF_S8W:CF_S8W + 16] = ((q16[None, :] == r[:, None] // 8)
                                   | (q16[None, :] == r[:, None] // 8 - 1)
                                   ).astype(np.float32)
    cf32[:, CF_S8TP:CF_S8TP + 16] = ((q16[None, :] == 15) & (r[:, None] < 8)).astype(np.float32)
    cf32[[0, 32, 64], CF_EYE] = 1.0

    # cbf16: 17-wide band constants (col 0 = strip pattern r<8)
    cbf16[:, CB_E0BX + 1:CB_E0BX + 17] = (q16[None, :] == r[:, None] // 8)
    cbf16[:, CB_E1BX + 1:CB_E1BX + 17] = (q16[None, :] == r[:, None] // 8 - 1)
    cbf16[:, CB_E1BX] = (r < 8)
    return cf32, cbf16


def _prep_inputs(x, mask, W, b_, v):
    """Host-side shard prep: core i gets batch i."""
    import ml_dtypes
    bf16 = ml_dtypes.bfloat16
    f8 = ml_dtypes.float8_e4m3

    # shared tensors
    w8 = np.ascontiguousarray(
        W.T.reshape(4, 128, 512).transpose(1, 0, 2).reshape(128, 4 * 512)
    ).astype(f8)
    bcol = np.ascontiguousarray(b_.reshape(4, 128).T).astype(np.float32)
    vcol = np.ascontiguousarray(v.reshape(4, 128).T).astype(bf16)

    maps = []
    for bi in range(B):
        xb = x[bi]                                    # [C, L] f32
        x8 = xb.astype(f8)
        xf8 = np.ascontiguousarray(
            x8.reshape(4, 128, 8, 512).transpose(1, 2, 0, 3).reshape(128, -1))
        xtb = np.ascontiguousarray(
            xb.T.astype(bf16).reshape(32, 128, 512).transpose(1, 0, 2)
            .reshape(128, -1))
        cf32, cbf16 = _build_host_constants(mask[bi])
        cf32[:, CF_BCOL:CF_BCOL + 4] = bcol
        cbf16[:, CB_VCOL:CB_VCOL + 4] = vcol
        maps.append({
            "xf8": xf8,
            "xt": xtb,
            "w8": w8,
            "cf32": cf32,
            "cbf16": cbf16.astype(bf16),
        })
    return maps


def kernel(x, mask, W, b, v):
    x = np.asarray(x, np.float32)
    mask = np.asarray(mask, bool)
    W = np.asarray(W, np.float32)
    b = np.asarray(b, np.float32)
    v = np.asarray(v, np.float32)

    from concourse.bass_utils import run_bass_kernel_spmd
    if "nc" not in _CACHE:
        _CACHE["nc"] = _build_bass()
    nc = _CACHE["nc"]

    in_maps = _prep_inputs(x, mask, W, b, v)
    res = run_bass_kernel_spmd(nc, in_maps, core_ids=list(range(8)))
    out = np.zeros((B, C, P), np.float32)
    for bi in range(B):
        outt = np.asarray(res.results[bi]["outt"], np.float32)   # [p, c]
        out[bi] = outt[:P].T
    return out


if __name__ == "__main__":
    import reference
    inputs = reference.setup_inputs()
    got = kernel(**{k: np.asarray(vv) for k, vv in inputs.items()})
    exp = np.asarray(reference.reference(**inputs))
    err = np.abs(got - exp).max() / np.abs(exp).max()
    print("scale-rel max err:", err)
